# revision 10
# baseline (speedup 1.0000x reference)
"""Trainium2 Bass kernel for nn_DinoGazeSpade (segment_reduce + repaint).

reference semantics:
  seg_feat = mask[:, ::14, ::14]                       # nearest-downsample to 28x28
  seg_avg[b, s, :] = mean of feat pixels with seg==s   # scatter_mean over B*128 segments
  out[b, :, hi, wi] = seg_avg[b, mask[b, hi, wi], :]   # repaint at full res

Sharding: 8 cores = 2 batches x 4 row-slices of the 392-row full-res output.
Each core computes its batch's seg_avg table (tiny) and paints its 98-row
slice. The paint is a one-hot(segment) x seg_avg matmul on the tensor engine,
which directly produces the channel-major output layout. Features are shipped
as exact bf16 hi/lo planes so the scatter-sum matmuls run at bf16 rate while
accumulating the exact fp32 values. The painted output is stored as fp16
(adds ~2^-11 relative rounding, far inside the 2e-2 gate), which halves the
dominant HBM write traffic; the paint weights are a single fp16 rounding of
seg_avg, so one matmul pass per tile suffices.

Pipeline layout (DMA-roofline bound: ~59 MB fp16 output per core):
  - whole mask prefetched in one early HWDGE DMA; one-hot masks are built on
    the otherwise-idle gpsimd engine (partition_broadcast + is_equal), keeping
    PE for paint matmuls only and DVE/ACT for the psum->fp16 copies
  - feature table loaded in 7 chunked HWDGE DMAs overlapped with the
    scatter-sum matmuls
  - all 8 PSUM banks rotate paint matmul outputs
"""

import numpy as np
import ml_dtypes
from contextlib import ExitStack

import concourse.bass as bass
import concourse.tile as tile
from concourse import bacc, mybir
from concourse.bass_utils import run_bass_kernel_spmd

# problem shape (hardcoded per contract)
B, C, Hp, Wp = 2, 768, 28, 28
Hi, Wi = 392, 392
S = 128                    # segments per image
N_CORES = 8
ROWS = Hi // 4             # 98 full-res rows per core
NPIX = ROWS * Wi           # 38416 pixels per core
NPATCH = Hp * Wp           # 784 patch pixels
PCHUNK = 112               # 784 = 7 * 112 patch-pixel chunks (partition dim)
NCH = NPATCH // PCHUNK     # 7 chunks
PTILE = 512                # paint pixel tile (one PSUM bank)
GROUP = 3 * PTILE          # 1536 pixels per paint group
NGROUP = NPIX // GROUP     # 25 full groups
REM = NPIX - NGROUP * GROUP  # 16 remainder pixels
CT = C // 128              # 6 channel tiles

f32 = mybir.dt.float32
bf16 = mybir.dt.bfloat16
fp16 = mybir.dt.float16
i32 = mybir.dt.int32

_CACHED_NC = None


def _build_nc():
    nc = bacc.Bacc()
    fpk_hbm = nc.dram_tensor("fpk", [PCHUNK, NCH, 2, C], bf16, kind="ExternalInput")
    pmk_hbm = nc.dram_tensor("pmk", [PCHUNK, NCH], f32, kind="ExternalInput")
    mask_hbm = nc.dram_tensor("mask", [1, NPIX], fp16, kind="ExternalInput")
    out_hbm = nc.dram_tensor("out", [C, NPIX], fp16, kind="ExternalOutput")

    with tile.TileContext(nc) as tc, ExitStack() as ctx:
        const = ctx.enter_context(tc.tile_pool(name="const", bufs=1))
        segp = ctx.enter_context(tc.tile_pool(name="segp", bufs=1))
        maskp = ctx.enter_context(tc.tile_pool(name="maskp", bufs=1))
        # paint-phase SBUF pools created BEFORE the scatter scratch pool so
        # the scatter pool's release doesn't alias them (early one-hot work
        # can then overlap the scatter phase)
        sbB = ctx.enter_context(tc.tile_pool(name="sbB", bufs=5))
        osb = ctx.enter_context(tc.tile_pool(name="osb", bufs=8))

        # ---- constants ----
        iota_pi = const.tile([128, 1], i32)           # partition index
        nc.gpsimd.iota(iota_pi[:], [[0, 1]], channel_multiplier=1)
        iota_pf = const.tile([128, 1], f32)
        nc.vector.tensor_copy(iota_pf[:], iota_pi[:])
        iota_ri = const.tile([128, 128], i32)         # free-dim index (same per partition)
        nc.gpsimd.iota(iota_ri[:], [[1, 128]], channel_multiplier=0)
        iota_rf = const.tile([128, 128], f32)
        nc.vector.tensor_copy(iota_rf[:], iota_ri[:])
        ones_col = const.tile([128, 1], bf16)
        nc.vector.memset(ones_col[:], 1.0)

        # ---- early input prefetch (HWDGE queues; SWDGE adds ~2us each) ----
        mask_sb = maskp.tile([1, NPIX], fp16)
        nc.scalar.dma_start(out=mask_sb[:], in_=mask_hbm[:, :])

        w16 = segp.tile([128, C], fp16)

        # ---- phase A: scatter-mean over patch pixels -> w16 [S=128, C] ----
        psA_cm = tc.tile_pool(name="psA", bufs=1, space="PSUM")
        with tc.tile_pool(name="sbA", bufs=1) as sbA, psA_cm as psA:
            sums0 = psA.tile([128, 384], f32, tag="sums0", name="sums0")
            sums1 = psA.tile([128, 384], f32, tag="sums1", name="sums1")
            cnt_ps = psA.tile([128, 1], f32, tag="cnt", name="cnt")
            pmk = sbA.tile([PCHUNK, NCH], f32, tag="pmk")
            nc.scalar.dma_start(out=pmk[:], in_=pmk_hbm[:, :])
            # chunked feature load: matmuls on chunk k overlap the DMA of
            # chunk k+1 (per-tile deps require separate tiles)
            fsb = [sbA.tile([PCHUNK, 2, C], bf16, tag=f"fsb{k}", name=f"fsb{k}")
                   for k in range(NCH)]
            for k in range(NCH):
                nc.sync.dma_start(out=fsb[k][:], in_=fpk_hbm[:, k, :, :])
            for k in range(NCH):
                oh = sbA.tile([PCHUNK, 128], bf16, tag="ohp")
                nc.vector.tensor_tensor(
                    out=oh[:], in0=iota_rf[0:PCHUNK, :],
                    in1=pmk[:, k:k + 1].to_broadcast([PCHUNK, 128]),
                    op=mybir.AluOpType.is_equal,
                )
                first, last = k == 0, k == NCH - 1
                for half, ps in ((0, sums0), (1, sums1)):
                    sl = slice(half * 384, (half + 1) * 384)
                    nc.tensor.matmul(ps[:], lhsT=oh[:], rhs=fsb[k][:, 0, sl],
                                     start=first, stop=False)
                    nc.tensor.matmul(ps[:], lhsT=oh[:], rhs=fsb[k][:, 1, sl],
                                     start=False, stop=last)
                nc.tensor.matmul(cnt_ps[:], lhsT=oh[:], rhs=ones_col[0:PCHUNK, :],
                                 start=first, stop=last)

            # r = 1 / max(cnt, 1); empty segments have sums == 0 so avg == 0.
            # The divide writes the fp16 paint weights directly.
            cnt_sb = sbA.tile([128, 1], f32)
            nc.vector.tensor_scalar_max(cnt_sb[:], cnt_ps[:], 1.0)
            rcp = sbA.tile([128, 1], f32)
            nc.vector.reciprocal(rcp[:], cnt_sb[:])
            nc.vector.tensor_scalar(
                out=w16[:, 0:384], in0=sums0[:], scalar1=rcp[:], scalar2=None,
                op0=mybir.AluOpType.mult,
            )
            nc.vector.tensor_scalar(
                out=w16[:, 384:768], in0=sums1[:], scalar1=rcp[:], scalar2=None,
                op0=mybir.AluOpType.mult,
            )

        # ---- phase B: paint full-res pixels ----
        psO = ctx.enter_context(tc.tile_pool(name="psO", bufs=8, space="PSUM"))

        def paint(pix0, sizes):
            # one group: pixels [pix0, pix0+sum(sizes)), one tile per size
            npx = sum(sizes)
            offs = [sum(sizes[:t]) for t in range(len(sizes))]
            # one-hot built on gpsimd: broadcast the mask row, compare with
            # the partition index
            mbc = sbB.tile([128, npx], fp16, tag="mbc", name="mbc")
            nc.gpsimd.partition_broadcast(mbc[:], mask_sb[0:1, pix0:pix0 + npx])
            oh = sbB.tile([128, npx], fp16, tag="ohb", name="ohb")
            nc.gpsimd.tensor_scalar(
                out=oh[:], in0=mbc[:], scalar1=iota_pf[:], scalar2=None,
                op0=mybir.AluOpType.is_equal,
            )
            for c in range(CT):
                ob = osb.tile([128, npx], fp16, tag="ob", name="ob")
                ops = [psO.tile([128, sz], f32, tag="op", name="op")
                       for sz in sizes]
                for t in range(len(sizes)):
                    nc.tensor.matmul(ops[t][:], lhsT=w16[:, c * 128:(c + 1) * 128],
                                     rhs=oh[:, offs[t]:offs[t] + sizes[t]],
                                     start=True, stop=True)
                for t in range(len(sizes)):
                    # split psum->sbuf convert-copies across DVE and ACT
                    dst = ob[:, offs[t]:offs[t] + sizes[t]]
                    if (c * len(sizes) + t) % 2 == 0:
                        nc.vector.tensor_copy(dst, ops[t][:])
                    else:
                        nc.scalar.copy(dst, ops[t][:])
                nc.sync.dma_start(
                    out=out_hbm[c * 128:(c + 1) * 128, pix0:pix0 + npx], in_=ob[:]
                )

        for g in range(NGROUP - 1):
            paint(g * GROUP, [PTILE] * 3)
        # last group absorbs the 16-pixel remainder as a 4th tile so the
        # final output DMA stays one large contiguous transfer per c-tile
        paint((NGROUP - 1) * GROUP, [PTILE] * 3 + ([REM] if REM else []))

    nc.compile()
    return nc


def _split_hilo(x):
    hi = x.astype(ml_dtypes.bfloat16)
    lo = (x - hi.astype(np.float32)).astype(ml_dtypes.bfloat16)
    return hi, lo


def make_in_maps(F_semantic_patches, segmentation_mask):
    F = np.asarray(F_semantic_patches, dtype=np.float32)
    M = np.asarray(segmentation_mask)
    in_maps = []
    for core in range(N_CORES):
        b, q = divmod(core, 4)
        feat = F[b].reshape(C, NPATCH).T                               # [784, 768]
        fhi, flo = _split_hilo(feat)
        # [p, k, plane, c] so one DMA lands chunk k on partitions
        fpk = np.ascontiguousarray(
            np.stack([fhi.reshape(NCH, PCHUNK, C), flo.reshape(NCH, PCHUNK, C)],
                     axis=2).transpose(1, 0, 2, 3)
        )
        pmk = np.ascontiguousarray(
            M[b, ::Hi // Hp, ::Wi // Wp].reshape(NCH, PCHUNK).T
        ).astype(np.float32)
        mask = np.ascontiguousarray(
            M[b, q * ROWS:(q + 1) * ROWS, :].reshape(1, NPIX)
        ).astype(np.float16)
        in_maps.append({"fpk": fpk, "pmk": pmk, "mask": mask})
    return in_maps


def kernel(F_semantic_patches: np.ndarray, segmentation_mask: np.ndarray) -> np.ndarray:
    global _CACHED_NC
    if _CACHED_NC is None:
        _CACHED_NC = _build_nc()
    nc = _CACHED_NC

    in_maps = make_in_maps(F_semantic_patches, segmentation_mask)

    res = run_bass_kernel_spmd(nc, in_maps, core_ids=list(range(N_CORES)))

    out = np.empty((B, C, Hi, Wi), dtype=np.float32)
    for core in range(N_CORES):
        b, q = divmod(core, 4)
        out[b, :, q * ROWS:(q + 1) * ROWS, :] = (
            res.results[core]["out"].reshape(C, ROWS, Wi)
        )
    return out


# revision 11
# speedup vs baseline: 3.0872x; 3.0872x over previous
"""Trainium2 Bass kernel for nn_DinoGazeSpade (segment_reduce + repaint).

reference semantics:
  seg_feat = mask[:, ::14, ::14]                       # nearest-downsample to 28x28
  seg_avg[b, s, :] = mean of feat pixels with seg==s   # scatter_mean over B*128 segments
  out[b, :, hi, wi] = seg_avg[b, mask[b, hi, wi], :]   # repaint at full res

Sharding: 8 cores = 2 batches x 4 row-slices of the 392-row full-res output.
Each core computes its batch's seg_avg table (tiny) and paints its 98-row
slice. The paint is a one-hot(segment) x seg_avg matmul on the tensor engine,
which directly produces the channel-major output layout.

The painted output is stored as fp16 (adds ~2^-11 relative rounding, far
inside the 2e-2 gate), which halves the dominant HBM write traffic; features
are shipped as fp16 too (products accumulate in fp32 on the PE), so both
phases are single-pass.

Pipeline layout (DMA-roofline bound: ~59 MB fp16 output per core):
  - whole mask prefetched in one early HWDGE DMA
  - per-group one-hot: partition_broadcast of the mask row on the otherwise
    idle gpsimd engine, then is_equal vs the partition index on DVE
  - feature table loaded in 7 chunked HWDGE DMAs overlapped with the
    scatter-sum matmuls
  - paint in 1024-px groups; all 8 PSUM banks rotate as 4 two-bank tiles;
    psum->fp16 convert-copies alternate whole-unit between DVE and ACT;
    output DMA issue alternates 5:1 between sync and scalar engines
"""

import numpy as np
from contextlib import ExitStack

import concourse.bass as bass
import concourse.tile as tile
from concourse import bacc, mybir
from concourse.bass_utils import run_bass_kernel_spmd

# problem shape (hardcoded per contract)
B, C, Hp, Wp = 2, 768, 28, 28
Hi, Wi = 392, 392
S = 128                    # segments per image
N_CORES = 8
ROWS = Hi // 4             # 98 full-res rows per core
NPIX = ROWS * Wi           # 38416 pixels per core
NPATCH = Hp * Wp           # 784 patch pixels
PCHUNK = 112               # 784 = 7 * 112 patch-pixel chunks (partition dim)
NCH = NPATCH // PCHUNK     # 7 chunks
PTILE = 512                # paint pixel tile (one PSUM bank)
GROUP = 2 * PTILE          # 1024 pixels per paint group (2-bank psum tile)
NGROUP = NPIX // GROUP     # 37 full groups
REM = NPIX - NGROUP * GROUP  # 528 remainder pixels (512 + 16)
CT = C // 128              # 6 channel tiles

f32 = mybir.dt.float32
bf16 = mybir.dt.bfloat16
fp16 = mybir.dt.float16
i32 = mybir.dt.int32

_CACHED_NC = None


def _build_nc():
    nc = bacc.Bacc()
    fpk_hbm = nc.dram_tensor("fpk", [PCHUNK, NCH, C], fp16, kind="ExternalInput")
    pmk_hbm = nc.dram_tensor("pmk", [PCHUNK, NCH], f32, kind="ExternalInput")
    mask_hbm = nc.dram_tensor("mask", [1, NPIX], fp16, kind="ExternalInput")
    out_hbm = nc.dram_tensor("out", [C, NPIX], fp16, kind="ExternalOutput")

    with tile.TileContext(nc) as tc, ExitStack() as ctx:
        const = ctx.enter_context(tc.tile_pool(name="const", bufs=1))
        segp = ctx.enter_context(tc.tile_pool(name="segp", bufs=1))
        maskp = ctx.enter_context(tc.tile_pool(name="maskp", bufs=1))
        # paint-phase SBUF pools created BEFORE the scatter scratch pool so
        # the scatter pool's release doesn't alias them (early one-hot work
        # can then overlap the scatter phase)
        sbB = ctx.enter_context(tc.tile_pool(name="sbB", bufs=5))
        osb = ctx.enter_context(tc.tile_pool(name="osb", bufs=8))

        # ---- constants ----
        iota_pi = const.tile([128, 1], i32)           # partition index
        nc.gpsimd.iota(iota_pi[:], [[0, 1]], channel_multiplier=1)
        iota_pf = const.tile([128, 1], f32)
        nc.vector.tensor_copy(iota_pf[:], iota_pi[:])
        iota_ri = const.tile([128, 128], i32)         # free-dim index (same per partition)
        nc.gpsimd.iota(iota_ri[:], [[1, 128]], channel_multiplier=0)
        iota_rf = const.tile([128, 128], f32)
        nc.vector.tensor_copy(iota_rf[:], iota_ri[:])
        ones_col = const.tile([128, 1], fp16)
        nc.vector.memset(ones_col[:], 1.0)

        # ---- early input prefetch (HWDGE queues; SWDGE adds ~2us each) ----
        mask_sb = maskp.tile([1, NPIX], fp16)
        nc.scalar.dma_start(out=mask_sb[:], in_=mask_hbm[:, :])

        w16 = segp.tile([128, C], fp16)

        # ---- phase A: scatter-mean over patch pixels -> w16 [S=128, C] ----
        psA_cm = tc.tile_pool(name="psA", bufs=1, space="PSUM")
        with tc.tile_pool(name="sbA", bufs=1) as sbA, psA_cm as psA:
            sums0 = psA.tile([128, 384], f32, tag="sums0", name="sums0")
            sums1 = psA.tile([128, 384], f32, tag="sums1", name="sums1")
            cnt_ps = psA.tile([128, 1], f32, tag="cnt", name="cnt")
            pmk = sbA.tile([PCHUNK, NCH], f32, tag="pmk")
            nc.scalar.dma_start(out=pmk[:], in_=pmk_hbm[:, :])
            # chunked feature load: matmuls on chunk k overlap the DMA of
            # chunk k+1 (per-tile deps require separate tiles)
            fsb = [sbA.tile([PCHUNK, C], fp16, tag=f"fsb{k}", name=f"fsb{k}")
                   for k in range(NCH)]
            for k in range(NCH):
                nc.sync.dma_start(out=fsb[k][:], in_=fpk_hbm[:, k, :])
            for k in range(NCH):
                oh = sbA.tile([PCHUNK, 128], fp16, tag="ohp")
                nc.vector.tensor_tensor(
                    out=oh[:], in0=iota_rf[0:PCHUNK, :],
                    in1=pmk[:, k:k + 1].to_broadcast([PCHUNK, 128]),
                    op=mybir.AluOpType.is_equal,
                )
                first, last = k == 0, k == NCH - 1
                nc.tensor.matmul(sums0[:], lhsT=oh[:], rhs=fsb[k][:, 0:384],
                                 start=first, stop=last)
                nc.tensor.matmul(sums1[:], lhsT=oh[:], rhs=fsb[k][:, 384:768],
                                 start=first, stop=last)
                nc.tensor.matmul(cnt_ps[:], lhsT=oh[:], rhs=ones_col[0:PCHUNK, :],
                                 start=first, stop=last)

            # r = 1 / max(cnt, 1); empty segments have sums == 0 so avg == 0.
            # The divide writes the fp16 paint weights directly.
            cnt_sb = sbA.tile([128, 1], f32)
            nc.vector.tensor_scalar_max(cnt_sb[:], cnt_ps[:], 1.0)
            rcp = sbA.tile([128, 1], f32)
            nc.vector.reciprocal(rcp[:], cnt_sb[:])
            nc.vector.tensor_scalar(
                out=w16[:, 0:384], in0=sums0[:], scalar1=rcp[:], scalar2=None,
                op0=mybir.AluOpType.mult,
            )
            nc.vector.tensor_scalar(
                out=w16[:, 384:768], in0=sums1[:], scalar1=rcp[:], scalar2=None,
                op0=mybir.AluOpType.mult,
            )

        # ---- phase B: paint full-res pixels ----
        psO = ctx.enter_context(tc.tile_pool(name="psO", bufs=4, space="PSUM"))
        unit = 0

        def paint(pix0, sizes):
            # one group: pixels [pix0, pix0+sum(sizes))
            nonlocal unit
            npx = sum(sizes)
            offs = [sum(sizes[:t]) for t in range(len(sizes))]
            # one-hot: gpsimd broadcasts the mask row across partitions,
            # DVE compares with the partition index
            mbc = sbB.tile([128, npx], fp16, tag="mbc", name="mbc")
            nc.gpsimd.partition_broadcast(mbc[:], mask_sb[0:1, pix0:pix0 + npx])
            oh = sbB.tile([128, npx], fp16, tag="ohb", name="ohb")
            nc.vector.tensor_scalar(
                out=oh[:], in0=mbc[:], scalar1=iota_pf[:], scalar2=None,
                op0=mybir.AluOpType.is_equal,
            )
            for c in range(CT):
                ob = osb.tile([128, npx], fp16, tag="ob", name="ob")
                op = psO.tile([128, npx], f32, tag="op", name="op")
                for t in range(len(sizes)):
                    sl = slice(offs[t], offs[t] + sizes[t])
                    nc.tensor.matmul(op[:, sl],
                                     lhsT=w16[:, c * 128:(c + 1) * 128],
                                     rhs=oh[:, sl], start=True, stop=True)
                # whole-unit psum->fp16 convert-copy, alternating DVE/ACT
                if unit % 2 == 0:
                    nc.vector.tensor_copy(ob[:], op[:])
                else:
                    nc.scalar.copy(ob[:], op[:])
                # output DMA issue costs ~650ns of engine time: mostly sync,
                # 1-in-6 on scalar to keep sync under the per-group budget
                eng = nc.scalar if unit % 6 == 5 else nc.sync
                eng.dma_start(
                    out=out_hbm[c * 128:(c + 1) * 128, pix0:pix0 + npx], in_=ob[:]
                )
                unit += 1

        for g in range(NGROUP):
            paint(g * GROUP, [PTILE] * 2)
        # remainder group: 512 + 16 pixels (one large contiguous DMA each)
        paint(NGROUP * GROUP, [PTILE, REM - PTILE])

    nc.compile()
    return nc


def make_in_maps(F_semantic_patches, segmentation_mask):
    F = np.asarray(F_semantic_patches, dtype=np.float32)
    M = np.asarray(segmentation_mask)
    in_maps = []
    for core in range(N_CORES):
        b, q = divmod(core, 4)
        feat = F[b].reshape(C, NPATCH).T.astype(np.float16)            # [784, 768]
        # [p, k, c] so one DMA lands chunk k on partitions
        fpk = np.ascontiguousarray(feat.reshape(NCH, PCHUNK, C).transpose(1, 0, 2))
        pmk = np.ascontiguousarray(
            M[b, ::Hi // Hp, ::Wi // Wp].reshape(NCH, PCHUNK).T
        ).astype(np.float32)
        mask = np.ascontiguousarray(
            M[b, q * ROWS:(q + 1) * ROWS, :].reshape(1, NPIX)
        ).astype(np.float16)
        in_maps.append({"fpk": fpk, "pmk": pmk, "mask": mask})
    return in_maps


def kernel(F_semantic_patches: np.ndarray, segmentation_mask: np.ndarray) -> np.ndarray:
    global _CACHED_NC
    if _CACHED_NC is None:
        _CACHED_NC = _build_nc()
    nc = _CACHED_NC

    in_maps = make_in_maps(F_semantic_patches, segmentation_mask)

    res = run_bass_kernel_spmd(nc, in_maps, core_ids=list(range(N_CORES)))

    out = np.empty((B, C, Hi, Wi), dtype=np.float32)
    for core in range(N_CORES):
        b, q = divmod(core, 4)
        out[b, :, q * ROWS:(q + 1) * ROWS, :] = (
            res.results[core]["out"].reshape(C, ROWS, Wi)
        )
    return out


# revision 12
# speedup vs baseline: 3.1373x; 1.0162x over previous
"""Trainium2 Bass kernel for nn_DinoGazeSpade (segment_reduce + repaint).

reference semantics:
  seg_feat = mask[:, ::14, ::14]                       # nearest-downsample to 28x28
  seg_avg[b, s, :] = mean of feat pixels with seg==s   # scatter_mean over B*128 segments
  out[b, :, hi, wi] = seg_avg[b, mask[b, hi, wi], :]   # repaint at full res

Sharding: 8 cores = 2 batches x 4 row-slices of the 392-row full-res output.
Each core computes its batch's seg_avg table (tiny) and paints its 98-row
slice. The paint is a one-hot(segment) x seg_avg matmul on the tensor engine,
which directly produces the channel-major output layout.

The painted output is stored as fp16 (adds ~2^-11 relative rounding, far
inside the 2e-2 gate), which halves the dominant HBM write traffic; features
are shipped as fp16 too (products accumulate in fp32 on the PE), so both
phases are single-pass.

Pipeline layout (DMA-roofline bound: ~59 MB fp16 output per core):
  - input DMAs issue first (pmk gates the phase-A chain), all on HWDGE
    queues; the whole mask is prefetched in one DMA
  - phase A builds all 7 chunk one-hots in ONE is_equal over a [112,7,128]
    iota, runs the count matmuls first (they need no features) so the
    reciprocal overlaps the sum matmuls, and produces the fp16 paint
    weights in two halves so painting starts after the first half
  - per-group one-hot: partition_broadcast of the mask row on the otherwise
    idle gpsimd engine, then is_equal vs the partition index on DVE
  - paint in pairs of 1024-px psum units (all 8 PSUM banks as 4 two-bank
    tiles); the pair shares one 2048-px output tile so output DMAs stay
    large; psum->fp16 convert-copies run on DVE (unit a) and ACT (unit b)
    in parallel; output DMA issue alternates 5:1 between sync and scalar
"""

import numpy as np
from contextlib import ExitStack

import concourse.bass as bass
import concourse.tile as tile
from concourse import bacc, mybir
from concourse.bass_utils import run_bass_kernel_spmd

# problem shape (hardcoded per contract)
B, C, Hp, Wp = 2, 768, 28, 28
Hi, Wi = 392, 392
S = 128                    # segments per image
N_CORES = 8
ROWS = Hi // 4             # 98 full-res rows per core
NPIX = ROWS * Wi           # 38416 pixels per core
NPATCH = Hp * Wp           # 784 patch pixels
PCHUNK = 112               # 784 = 7 * 112 patch-pixel chunks (partition dim)
NCH = NPATCH // PCHUNK     # 7 chunks
PTILE = 512                # paint pixel tile (one PSUM bank)
GROUP = 2 * PTILE          # 1024 pixels per psum unit (2-bank psum tile)
NPAIR = 19                 # 19 pairs of units cover 38416 px (last pair short)
CT = C // 128              # 6 channel tiles

f32 = mybir.dt.float32
bf16 = mybir.dt.bfloat16
fp16 = mybir.dt.float16
i32 = mybir.dt.int32

_CACHED_NC = None


def _build_nc():
    nc = bacc.Bacc()
    fpk_hbm = nc.dram_tensor("fpk", [PCHUNK, NCH, C], fp16, kind="ExternalInput")
    pmk_hbm = nc.dram_tensor("pmk", [PCHUNK, NCH], f32, kind="ExternalInput")
    mask_hbm = nc.dram_tensor("mask", [1, NPIX], fp16, kind="ExternalInput")
    out_hbm = nc.dram_tensor("out", [C, NPIX], fp16, kind="ExternalOutput")

    with tile.TileContext(nc) as tc, ExitStack() as ctx:
        const = ctx.enter_context(tc.tile_pool(name="const", bufs=1))
        segp = ctx.enter_context(tc.tile_pool(name="segp", bufs=1))
        maskp = ctx.enter_context(tc.tile_pool(name="maskp", bufs=1))
        # paint-phase SBUF pools created BEFORE the scatter scratch pool so
        # the scatter pool's release doesn't alias them (early one-hot work
        # can then overlap the scatter phase)
        sbB = ctx.enter_context(tc.tile_pool(name="sbB", bufs=6))
        osb = ctx.enter_context(tc.tile_pool(name="osb", bufs=8))
        sbA_cm = tc.tile_pool(name="sbA", bufs=1)
        psA_cm = tc.tile_pool(name="psA", bufs=1, space="PSUM")

        with sbA_cm as sbA, psA_cm as psA:
            # ---- input DMAs first: pmk gates the phase-A one-hot chain ----
            pmk = sbA.tile([PCHUNK, NCH], f32, tag="pmk")
            nc.scalar.dma_start(out=pmk[:], in_=pmk_hbm[:, :])
            # chunked feature load: matmuls on chunk k overlap the DMA of
            # chunk k+1 (per-tile deps require separate tiles)
            fsb = [sbA.tile([PCHUNK, C], fp16, tag=f"fsb{k}", name=f"fsb{k}")
                   for k in range(NCH)]
            for k in range(NCH):
                nc.sync.dma_start(out=fsb[k][:], in_=fpk_hbm[:, k, :])
            mask_sb = maskp.tile([1, NPIX], fp16)
            nc.scalar.dma_start(out=mask_sb[:], in_=mask_hbm[:, :])

            # ---- constants ----
            iota_pi = const.tile([128, 1], i32)           # partition index
            nc.gpsimd.iota(iota_pi[:], [[0, 1]], channel_multiplier=1)
            iota_pf = const.tile([128, 1], f32)
            nc.vector.tensor_copy(iota_pf[:], iota_pi[:])
            # [*, k, s] = s: compare target for all 7 chunk one-hots at once
            iota_all = const.tile([128, NCH, 128], f32)
            nc.gpsimd.iota(iota_all[:], [[0, NCH], [1, 128]],
                           channel_multiplier=0,
                           allow_small_or_imprecise_dtypes=True)
            ones_col = const.tile([128, 1], fp16)
            nc.vector.memset(ones_col[:], 1.0)

            w16a = segp.tile([128, 384], fp16)
            w16b = segp.tile([128, 384], fp16)

            # ---- phase A: scatter-mean over patch pixels -> w16 [S=128, C] ----
            sums0 = psA.tile([128, 384], f32, tag="sums0", name="sums0")
            sums1 = psA.tile([128, 384], f32, tag="sums1", name="sums1")
            cnt_ps = psA.tile([128, 1], f32, tag="cnt", name="cnt")

            oh_all = sbA.tile([128, NCH, 128], fp16, tag="ohall", name="ohall")
            nc.vector.tensor_tensor(
                out=oh_all[0:PCHUNK], in0=iota_all[0:PCHUNK],
                in1=pmk[:, :].to_broadcast([PCHUNK, NCH, 128]),
                op=mybir.AluOpType.is_equal,
            )
            # counts need no features: run them first so max/reciprocal
            # overlap the sum matmuls
            for k in range(NCH):
                nc.tensor.matmul(cnt_ps[:], lhsT=oh_all[0:PCHUNK, k, :],
                                 rhs=ones_col[0:PCHUNK, :],
                                 start=k == 0, stop=k == NCH - 1)
            cnt_sb = sbA.tile([128, 1], f32)
            nc.vector.tensor_scalar_max(cnt_sb[:], cnt_ps[:], 1.0)
            rcp = sbA.tile([128, 1], f32)
            nc.vector.reciprocal(rcp[:], cnt_sb[:])
            for k in range(NCH):
                first, last = k == 0, k == NCH - 1
                nc.tensor.matmul(sums0[:], lhsT=oh_all[0:PCHUNK, k, :],
                                 rhs=fsb[k][:, 0:384], start=first, stop=last)
                nc.tensor.matmul(sums1[:], lhsT=oh_all[0:PCHUNK, k, :],
                                 rhs=fsb[k][:, 384:768], start=first, stop=last)
            # divide writes the fp16 paint weights directly (empty segments
            # have sums == 0 so avg == 0); two halves so ctiles 0-2 can
            # paint before sums1 is divided
            nc.vector.tensor_scalar(
                out=w16a[:], in0=sums0[:], scalar1=rcp[:], scalar2=None,
                op0=mybir.AluOpType.mult,
            )
            nc.vector.tensor_scalar(
                out=w16b[:], in0=sums1[:], scalar1=rcp[:], scalar2=None,
                op0=mybir.AluOpType.mult,
            )

        # ---- phase B: paint full-res pixels ----
        psO = ctx.enter_context(tc.tile_pool(name="psO", bufs=4, space="PSUM"))

        def paint_pair(pix0, sizes_a, sizes_b):
            # two psum units (<=1024 px each) sharing one output tile
            halves = []
            off = 0
            for sizes in (sizes_a, sizes_b):
                n = sum(sizes)
                # one-hot: gpsimd broadcasts the mask row across partitions,
                # DVE compares with the partition index
                mbc = sbB.tile([128, n], fp16, tag="mbc", name="mbc")
                nc.gpsimd.partition_broadcast(
                    mbc[:], mask_sb[0:1, pix0 + off:pix0 + off + n])
                oh = sbB.tile([128, n], fp16, tag="ohb", name="ohb")
                nc.vector.tensor_scalar(
                    out=oh[:], in0=mbc[:], scalar1=iota_pf[:], scalar2=None,
                    op0=mybir.AluOpType.is_equal,
                )
                halves.append((off, sizes, n, oh))
                off += n
            npx = off
            for c in range(CT):
                wsl_t = w16a if c < 3 else w16b
                wsl = wsl_t[:, (c % 3) * 128:(c % 3 + 1) * 128]
                ob = osb.tile([128, npx], fp16, tag="ob", name="ob")
                for idx, (hoff, sizes, n, oh) in enumerate(halves):
                    op = psO.tile([128, n], f32, tag="op", name="op")
                    o2 = 0
                    for sz in sizes:
                        nc.tensor.matmul(op[:, o2:o2 + sz], lhsT=wsl,
                                         rhs=oh[:, o2:o2 + sz],
                                         start=True, stop=True)
                        o2 += sz
                    # psum->fp16 convert-copies: unit a on DVE, unit b on ACT
                    dst = ob[:, hoff:hoff + n]
                    if idx == 0:
                        nc.vector.tensor_copy(dst, op[:])
                    else:
                        nc.scalar.copy(dst, op[:])
                # output DMA issue costs ~650ns of engine time: mostly sync,
                # 1-in-6 on scalar to keep sync under the per-pair budget
                eng = nc.scalar if c == 5 else nc.sync
                eng.dma_start(
                    out=out_hbm[c * 128:(c + 1) * 128, pix0:pix0 + npx],
                    in_=ob[:],
                )

        for p in range(NPAIR - 1):
            paint_pair(p * 2 * GROUP, [PTILE] * 2, [PTILE] * 2)
        # last pair: 1024 + 528 pixels (512 + 16 remainder in unit b)
        paint_pair((NPAIR - 1) * 2 * GROUP, [PTILE] * 2, [PTILE, 16])

    nc.compile()
    return nc


def make_in_maps(F_semantic_patches, segmentation_mask):
    F = np.asarray(F_semantic_patches, dtype=np.float32)
    M = np.asarray(segmentation_mask)
    in_maps = []
    for core in range(N_CORES):
        b, q = divmod(core, 4)
        feat = F[b].reshape(C, NPATCH).T.astype(np.float16)            # [784, 768]
        # [p, k, c] so one DMA lands chunk k on partitions
        fpk = np.ascontiguousarray(feat.reshape(NCH, PCHUNK, C).transpose(1, 0, 2))
        pmk = np.ascontiguousarray(
            M[b, ::Hi // Hp, ::Wi // Wp].reshape(NCH, PCHUNK).T
        ).astype(np.float32)
        mask = np.ascontiguousarray(
            M[b, q * ROWS:(q + 1) * ROWS, :].reshape(1, NPIX)
        ).astype(np.float16)
        in_maps.append({"fpk": fpk, "pmk": pmk, "mask": mask})
    return in_maps


def kernel(F_semantic_patches: np.ndarray, segmentation_mask: np.ndarray) -> np.ndarray:
    global _CACHED_NC
    if _CACHED_NC is None:
        _CACHED_NC = _build_nc()
    nc = _CACHED_NC

    in_maps = make_in_maps(F_semantic_patches, segmentation_mask)

    res = run_bass_kernel_spmd(nc, in_maps, core_ids=list(range(N_CORES)))

    out = np.empty((B, C, Hi, Wi), dtype=np.float32)
    for core in range(N_CORES):
        b, q = divmod(core, 4)
        out[b, :, q * ROWS:(q + 1) * ROWS, :] = (
            res.results[core]["out"].reshape(C, ROWS, Wi)
        )
    return out
